# revision 2
# baseline (speedup 1.0000x reference)
"""Trainium2 Bass kernel for an attention block with softmax over the QUERY axis.

Reference computation (per batch b):
    Q = x_b @ Wq^T + bq ; K = x_b @ Wk^T + bk ; V = x_b @ Wv^T + bv
    S = Q @ K^T / sqrt(256)
    attn = softmax(S, axis over queries)      # couples rows, not columns
    out_b = attn @ V

Sharding over 8 NeuronCores: core m handles batch b = m // 2 and the
key/value half h = m % 2.  The host passes x_b^T with the query axis
rotated so that the core's 2048 keys are always columns 0:2048; each core
holds the full query range for its batch, so the softmax over queries is
fully local.  Each core produces a partial output (sum over its 2048
keys); the host rotates back and adds the two partials per batch.  No
collectives are required.

On-core dataflow (all matmul operands bf16, fp32 accumulation):
    xT  (256d, 4096s)   host-transposed input, d on partitions
    QT  (256e, 4096q) = Wq @ x^T        KT (256e, 2048k) = Wk @ x_h^T
    V   (2048k, 256e) = x_h @ Wv^T      (natural layout, k on partitions)
    ST  (k, q) tiles  = KT^T slices @ QT (scores transposed, PSUM)
    e   = exp(ST/16)  on ACT, row sums via the activation accumulator
                      (no max-subtraction: |S/16| < ~2.2 for these inputs)
    V'  = V / s_k     per-partition scale (in place, DVE)
    out = e^T @ V'    k-split in half: the first half's matmuls are
                      interleaved under the ACT-bound scores phase.
"""

import numpy as np
import ml_dtypes

import concourse.bass as bass
import concourse.tile as tile
from concourse import bacc, mybir
from concourse.bass_utils import run_bass_kernel_spmd

BF16 = ml_dtypes.bfloat16
F32 = mybir.dt.float32
BF = mybir.dt.bfloat16

B, S, D = 4, 4096, 256
NCORES = 8
KH = S // 2          # 2048 keys per core
NKT = KH // 128      # 16 key tiles
NQT = S // 128       # 32 query tiles
NK1 = NKT // 2       # key tiles in the interleaved AV half

EXP = mybir.ActivationFunctionType.Exp
AX = mybir.AxisListType.X


def _emit(tc, xT, wqT, wkT, wvT, bqc, bkc, bvr, out):
    nc = tc.nc

    with tc.tile_pool(name="const", bufs=1) as cpool, \
         tc.tile_pool(name="big", bufs=1) as bpool, \
         tc.tile_pool(name="work", bufs=4) as wpool:

        # ---- input loads: small tensors first, then xT in column halves ----
        def load_pair(name, src, cols):
            ts = []
            for i in range(2):
                t = cpool.tile([128, cols], src.dtype, name=f"{name}{i}",
                               tag=f"{name}{i}")
                ts.append(t)
            return ts

        wq_sb = load_pair("wq", wqT, D)
        wk_sb = load_pair("wk", wkT, D)
        wv_sb = load_pair("wv", wvT, D)
        bq_sb = load_pair("bq", bqc, 1)
        bk_sb = load_pair("bk", bkc, 1)
        for tpair, src in ((wq_sb, wqT), (wk_sb, wkT), (wv_sb, wvT),
                           (bq_sb, bqc), (bk_sb, bkc)):
            for i in range(2):
                nc.sync.dma_start(tpair[i], src[128 * i:128 * (i + 1), :])
        bv_sb = cpool.tile([1, D], BF, name="bv", tag="bv")
        nc.sync.dma_start(bv_sb, bvr)
        ones = cpool.tile([1, 128], BF, name="ones", tag="ones")
        nc.vector.memset(ones, 1.0)

        xT_sb = [cpool.tile([128, S], BF, name=f"xTsb{i}", tag=f"xTsb{i}")
                 for i in range(2)]
        for i in range(2):                       # key/query half first
            nc.sync.dma_start(xT_sb[i][:, 0:KH], xT[128 * i:128 * (i + 1), 0:KH])
        for i in range(2):
            nc.sync.dma_start(xT_sb[i][:, KH:S], xT[128 * i:128 * (i + 1), KH:S])

        # ---- persistent tiles ----
        QT_sb = [bpool.tile([128, S], BF, name=f"QT{i}", tag=f"QT{i}")
                 for i in range(2)]
        KT_sb = [bpool.tile([128, KH], BF, name=f"KT{i}", tag=f"KT{i}")
                 for i in range(2)]
        Vb_sb = [bpool.tile([128, D], BF, name=f"Vb{k}", tag=f"Vb{k}")
                 for k in range(NKT)]
        e_sb = [bpool.tile([128, S], BF, name=f"e{k}", tag=f"e{k}")
                for k in range(NKT)]
        part_sb = [bpool.tile([128, D], BF, name=f"pt{j}", tag=f"pt{j}")
                   for j in range(NQT)]
        bvb_sb = cpool.tile([128, D], BF, name="bvb", tag="bvb")

        # ---- emission helpers (each works inside a (128,2048) psum slot) ----
        def emit_bv_bcast(slot):
            po = slot[:, 0:D]
            nc.tensor.matmul(po, ones, bv_sb, start=True, stop=True)
            nc.vector.tensor_copy(bvb_sb, po)

        def emit_kt_block(slot, blk):
            # KT columns [512*blk, 512*(blk+1)) for both e-tiles
            for i in range(2):
                sub = slot[:, 1024 * i:1024 * i + 512]
                cs = slice(512 * blk, 512 * (blk + 1))
                nc.tensor.matmul(sub, wk_sb[0][:, 128 * i:128 * (i + 1)],
                                 xT_sb[0][:, cs], start=True, stop=False)
                nc.tensor.matmul(sub, wk_sb[1][:, 128 * i:128 * (i + 1)],
                                 xT_sb[1][:, cs], start=False, stop=True)
                nc.vector.tensor_scalar_add(KT_sb[i][:, cs], sub, bk_sb[i])

        def emit_qt_group(slot, sub_idx, i, qb):
            sub = slot[:, 512 * sub_idx:512 * (sub_idx + 1)]
            cs = slice(512 * qb, 512 * (qb + 1))
            nc.tensor.matmul(sub, wq_sb[0][:, 128 * i:128 * (i + 1)],
                             xT_sb[0][:, cs], start=True, stop=False)
            nc.tensor.matmul(sub, wq_sb[1][:, 128 * i:128 * (i + 1)],
                             xT_sb[1][:, cs], start=False, stop=True)
            nc.vector.tensor_scalar_add(QT_sb[i][:, cs], sub, bq_sb[i])

        def emit_v_group(slot, sub_idx, k):
            sub = slot[:, 256 * sub_idx:256 * (sub_idx + 1)]
            ks = slice(128 * k, 128 * (k + 1))
            nc.tensor.matmul(sub, xT_sb[0][:, ks], wv_sb[0],
                             start=True, stop=False)
            nc.tensor.matmul(sub, xT_sb[1][:, ks], wv_sb[1],
                             start=False, stop=True)
            nc.vector.tensor_tensor(Vb_sb[k], sub, bvb_sb,
                                    op=mybir.AluOpType.add)

        def emit_scores_slot(slot, k, q0, width, sparts, sidx):
            # slot[:, 0:width] = KT[:, ktile k]^T @ QT[:, q0:q0+width]
            for g in range(width // 512):
                sub = slot[:, 512 * g:512 * (g + 1)]
                qs = slice(q0 + 512 * g, q0 + 512 * (g + 1))
                nc.tensor.matmul(sub, KT_sb[0][:, 128 * k:128 * (k + 1)],
                                 QT_sb[0][:, qs], start=True, stop=False)
                nc.tensor.matmul(sub, KT_sb[1][:, 128 * k:128 * (k + 1)],
                                 QT_sb[1][:, qs], start=False, stop=True)
            nc.scalar.activation(e_sb[k][:, q0:q0 + width], slot[:, 0:width],
                                 EXP, scale=1.0 / 16.0,
                                 accum_out=sparts[:, sidx:sidx + 1])

        def emit_row_scale(k, sparts):
            ssum = wpool.tile([128, 1], F32, name="ssum", tag="ssum")
            nc.vector.reduce_sum(ssum, sparts, axis=AX)
            rs = wpool.tile([128, 1], F32, name="rs", tag="rs")
            nc.vector.reciprocal(rs, ssum)
            nc.vector.tensor_scalar_mul(Vb_sb[k], Vb_sb[k], rs)

        # ================= phase 0 + 1a: big psum slots =================
        with tc.tile_pool(name="psa", bufs=2, space="PSUM") as psa:
            def slot_a():
                return psa.tile([128, 2048], F32, name="psat", tag="psat")

            s = slot_a()
            emit_bv_bcast(s)
            s = slot_a()
            emit_kt_block(s, 0)            # uses subranges 0 and 1024
            s = slot_a()
            emit_kt_block(s, 1)
            s = slot_a()
            for v in range(8):
                emit_v_group(s, v, v)      # V k-tiles 0..7
            for qb_pair in range(4):       # QT blocks, 4 groups per slot
                s = slot_a()
                for t in range(2):
                    qb = 2 * qb_pair + t
                    for i in range(2):
                        emit_qt_group(s, 2 * t + i, i, qb)

            # phase 1a: scores k-tiles 0..7, deferred projections as filler
            for k in range(NK1):
                if k == 1:
                    s = slot_a()
                    emit_kt_block(s, 2)
                if k == 2:
                    s = slot_a()
                    emit_kt_block(s, 3)
                if k == 3:
                    s = slot_a()
                    for v in range(8):
                        emit_v_group(s, v, 8 + v)   # V k-tiles 8..15
                sparts = wpool.tile([128, 2], F32, name="sparts", tag="sparts")
                for half in range(2):
                    s = slot_a()
                    emit_scores_slot(s, k, 2048 * half, 2048, sparts, half)
                emit_row_scale(k, sparts)

        # ============ phase 1b + 2: small slots + AV accumulators ============
        with tc.tile_pool(name="psb", bufs=2, space="PSUM") as psb, \
             tc.tile_pool(name="psav", bufs=4, space="PSUM") as psav:

            def emit_av(j, krange, accumulate_part):
                pa = psav.tile([128, D], F32, name="psavt", tag="psavt")
                for n, k in enumerate(krange):
                    nc.tensor.matmul(pa, e_sb[k][:, 128 * j:128 * (j + 1)],
                                     Vb_sb[k], start=(n == 0),
                                     stop=(n == len(krange) - 1))
                if not accumulate_part:
                    nc.vector.tensor_copy(part_sb[j], pa)
                else:
                    ot = wpool.tile([128, D], F32, name="osb", tag="osb")
                    nc.vector.tensor_tensor(ot, pa, part_sb[j],
                                            op=mybir.AluOpType.add)
                    nc.sync.dma_start(out[128 * j:128 * (j + 1), :], ot)

            for k in range(NK1, NKT):
                sparts = wpool.tile([128, 4], F32, name="sparts", tag="sparts")
                for quarter in range(4):
                    s = psb.tile([128, 1024], F32, name="psbt", tag="psbt")
                    emit_scores_slot(s, k, 1024 * quarter, 1024, sparts, quarter)
                emit_row_scale(k, sparts)
                # AV over the first key half for 4 query tiles (PE filler)
                for j in range(4 * (k - NK1), 4 * (k - NK1) + 4):
                    emit_av(j, range(NK1), accumulate_part=False)

            for j in range(NQT):
                emit_av(j, range(NK1, NKT), accumulate_part=True)


def build():
    nc = bacc.Bacc("TRN2", target_bir_lowering=False, debug=False)
    xT = nc.dram_tensor("xT", [D, S], BF, kind="ExternalInput").ap()
    wqT = nc.dram_tensor("wqT", [D, D], BF, kind="ExternalInput").ap()
    wkT = nc.dram_tensor("wkT", [D, D], BF, kind="ExternalInput").ap()
    wvT = nc.dram_tensor("wvT", [D, D], BF, kind="ExternalInput").ap()
    bqc = nc.dram_tensor("bqc", [D, 1], F32, kind="ExternalInput").ap()
    bkc = nc.dram_tensor("bkc", [D, 1], F32, kind="ExternalInput").ap()
    bvr = nc.dram_tensor("bvr", [1, D], BF, kind="ExternalInput").ap()
    out = nc.dram_tensor("out", [S, D], F32, kind="ExternalOutput").ap()

    with tile.TileContext(nc) as tc:
        _emit(tc, xT, wqT, wkT, wvT, bqc, bkc, bvr, out)
    nc.compile()
    return nc


_NC = None


def _get_nc():
    global _NC
    if _NC is None:
        _NC = build()
    return _NC


def make_in_maps(x, Wq, bq, Wk, bk, Wv, bv):
    wq = np.ascontiguousarray(Wq.T).astype(BF16)
    wk = np.ascontiguousarray(Wk.T).astype(BF16)
    wv = np.ascontiguousarray(Wv.T).astype(BF16)
    bqc = np.asarray(bq, np.float32).reshape(D, 1)
    bkc = np.asarray(bk, np.float32).reshape(D, 1)
    bvr = np.asarray(bv).reshape(1, D).astype(BF16)
    in_maps = []
    for core in range(NCORES):
        b, h = divmod(core, 2)
        xTb = np.asarray(x[b]).T.astype(BF16)
        if h:  # rotate so this core's keys are always columns 0:KH
            xTb = np.concatenate([xTb[:, KH:], xTb[:, :KH]], axis=1)
        in_maps.append({
            "xT": np.ascontiguousarray(xTb),
            "wqT": wq, "wkT": wk, "wvT": wv,
            "bqc": bqc, "bkc": bkc, "bvr": bvr,
        })
    return in_maps


def run(x, Wq, bq, Wk, bk, Wv, bv, trace=False):
    """Run on the 8 cores; returns (full_output, BassKernelResults)."""
    nc = _get_nc()
    in_maps = make_in_maps(x, Wq, bq, Wk, bk, Wv, bv)
    res = run_bass_kernel_spmd(nc, in_maps, core_ids=list(range(NCORES)),
                               trace=trace)
    parts = []
    for core in range(NCORES):
        p = res.results[core]["out"]
        if core % 2:  # undo the query rotation
            p = np.concatenate([p[KH:], p[:KH]], axis=0)
        parts.append(p)
    full = np.stack([parts[2 * b] + parts[2 * b + 1] for b in range(B)], axis=0)
    return full.astype(np.float32), res


def kernel(x, Wq, bq, Wk, bk, Wv, bv):
    full, _ = run(x, Wq, bq, Wk, bk, Wv, bv, trace=False)
    return full


# revision 6
# speedup vs baseline: 1.2953x; 1.2953x over previous
"""Trainium2 Bass kernel for an attention block with softmax over the QUERY axis.

Reference computation (per batch b):
    Q = x_b @ Wq^T + bq ; K = x_b @ Wk^T + bk ; V = x_b @ Wv^T + bv
    S = Q @ K^T / sqrt(256)
    attn = softmax(S, axis over queries)      # couples rows, not columns
    out_b = attn @ V

Sharding over 8 NeuronCores: core m handles batch b = m // 2 and the
key/value half h = m % 2.  The host passes x_b^T with the query axis
rotated so that the core's 2048 keys are always columns 0:2048; each core
holds the full query range for its batch, so the softmax over queries is
fully local.  Each core produces a partial output (sum over its 2048
keys); the host rotates back and adds the two partials per batch.  No
collectives are required.

On-core dataflow (all matmul operands bf16, fp32 accumulation):
    xT  (256d, 4096s)   host-transposed input, d on partitions
    QT  (256e, 4096q) = Wq @ x^T        KT (256e, 2048k) = Wk @ x_h^T
    V   (2048k, 256e) = x_h @ Wv^T      (natural layout, k on partitions)
    ST  (k, q) tiles  = KT^T slices @ QT (scores transposed, PSUM)
    e   = exp(ST/16)  on ACT, row sums via the activation accumulator
                      (no max-subtraction: |S/16| < ~2.2 for these inputs)
    V'  = V / s_k     per-partition scale (in place, DVE)
    out = e^T @ V'    k-split in half: the first half's matmuls are
                      interleaved under the ACT-bound scores phase.
"""

import numpy as np
import ml_dtypes

import concourse.bass as bass
import concourse.tile as tile
from concourse import bacc, mybir
from concourse.bass_utils import run_bass_kernel_spmd

BF16 = ml_dtypes.bfloat16
F32 = mybir.dt.float32
BF = mybir.dt.bfloat16

B, S, D = 4, 4096, 256
NCORES = 8
KH = S // 2          # 2048 keys per core
NKT = KH // 128      # 16 key tiles
NQT = S // 128       # 32 query tiles
NK1 = NKT // 2       # key tiles in the interleaved AV half

EXP = mybir.ActivationFunctionType.Exp
IDENT = mybir.ActivationFunctionType.Identity
AX = mybir.AxisListType.X
ADD = mybir.AluOpType.add


def _emit(tc, xT, wpT, bpack, bvr, out):
    nc = tc.nc

    with tc.tile_pool(name="const", bufs=1) as cpool, \
         tc.tile_pool(name="big", bufs=1) as bpool, \
         tc.tile_pool(name="work", bufs=4) as wpool:

        # ---- input loads: packed, few DMAs ----
        w_sb = [cpool.tile([128, 3 * D], BF, name=f"wsb{i}", tag=f"wsb{i}")
                for i in range(2)]
        bqk_sb = [cpool.tile([128, 2], F32, name=f"bqk{i}", tag=f"bqk{i}")
                  for i in range(2)]
        for i in range(2):
            nc.sync.dma_start(w_sb[i], wpT[128 * i:128 * (i + 1), :])
            nc.sync.dma_start(bqk_sb[i], bpack[128 * i:128 * (i + 1), :])
        bv_sb = cpool.tile([1, D], BF, name="bv", tag="bv")
        nc.sync.dma_start(bv_sb, bvr)
        ones = cpool.tile([1, 128], BF, name="ones", tag="ones")
        nc.vector.memset(ones, 1.0)

        # xT as one (128, 2*4096) tile: [:, 0:S] = d-rows 0:128, [:, S:2S] =
        # d-rows 128:256.  Loaded in two column-halves (keys first) so the
        # projections can start before the full tensor arrives.
        xT_sb = cpool.tile([128, 2 * S], BF, name="xTsb", tag="xTsb")
        xt_out = xT_sb.rearrange("p (t s) -> p t s", t=2)
        xt_in = xT.rearrange("(t p) s -> p t s", p=128)
        nc.sync.dma_start(xt_out[:, :, 0:KH], xt_in[:, :, 0:KH])
        nc.sync.dma_start(xt_out[:, :, KH:S], xt_in[:, :, KH:S])
        xTs = [xT_sb[:, 0:S], xT_sb[:, S:2 * S]]

        def wq(i):
            return w_sb[i][:, 0:D]

        def wk(i):
            return w_sb[i][:, D:2 * D]

        def wv(i):
            return w_sb[i][:, 2 * D:3 * D]

        # ---- persistent tiles ----
        QT_sb = [bpool.tile([128, S], BF, name=f"QT{i}", tag=f"QT{i}")
                 for i in range(2)]
        KT_sb = [bpool.tile([128, KH], BF, name=f"KT{i}", tag=f"KT{i}")
                 for i in range(2)]
        Vb_sb = [bpool.tile([128, D], BF, name=f"Vb{k}", tag=f"Vb{k}")
                 for k in range(NKT)]
        e_sb = [bpool.tile([128, S], BF, name=f"e{k}", tag=f"e{k}")
                for k in range(NKT)]
        part_sb = [bpool.tile([128, D], BF, name=f"pt{j}", tag=f"pt{j}")
                   for j in range(NQT)]
        bvb_sb = cpool.tile([128, D], BF, name="bvb", tag="bvb")

        # ================= phase 0: projections (small-slot ring) ==========
        with tc.tile_pool(name="ps0", bufs=8, space="PSUM") as ps0:
            def slot0():
                return ps0.tile([128, 512], F32, name="ps0t", tag="ps0t")

            pt = slot0()
            nc.tensor.matmul(pt[:, 0:D], ones, bv_sb, start=True, stop=True)
            nc.vector.tensor_copy(bvb_sb, pt[:, 0:D])

            # KT then QT: two d-tile matmuls per 512-column group; bias added
            # on the copyback (DVE for e-tile 0, ACT for e-tile 1).
            for dst, wsel, bcol, ncols in ((KT_sb, wk, 1, KH),
                                           (QT_sb, wq, 0, S)):
                for qb in range(ncols // 512):
                    for i in range(2):
                        pt = slot0()
                        cs = slice(512 * qb, 512 * (qb + 1))
                        nc.tensor.matmul(pt, wsel(0)[:, 128 * i:128 * (i + 1)],
                                         xTs[0][:, cs], start=True, stop=False)
                        nc.tensor.matmul(pt, wsel(1)[:, 128 * i:128 * (i + 1)],
                                         xTs[1][:, cs], start=False, stop=True)
                        bias = bqk_sb[i][:, bcol:bcol + 1]
                        if i == 0:
                            nc.vector.tensor_scalar_add(dst[i][:, cs], pt, bias)
                        else:
                            nc.scalar.activation(dst[i][:, cs], pt, IDENT,
                                                 bias=bias)
            # V natural layout; bv added via the broadcast tile on DVE.
            for k in range(NKT):
                pt = slot0()
                po = pt[:, 0:D]
                ks = slice(128 * k, 128 * (k + 1))
                nc.tensor.matmul(po, xTs[0][:, ks], wv(0),
                                 start=True, stop=False)
                nc.tensor.matmul(po, xTs[1][:, ks], wv(1),
                                 start=False, stop=True)
                nc.vector.tensor_tensor(Vb_sb[k], po, bvb_sb, op=ADD)

        # ---- shared helpers for the scores phases ----
        def emit_scores_slot(slot, k, q0, width, sparts, sidx):
            for g in range(width // 512):
                sub = slot[:, 512 * g:512 * (g + 1)]
                qs = slice(q0 + 512 * g, q0 + 512 * (g + 1))
                nc.tensor.matmul(sub, KT_sb[0][:, 128 * k:128 * (k + 1)],
                                 QT_sb[0][:, qs], start=True, stop=False)
                nc.tensor.matmul(sub, KT_sb[1][:, 128 * k:128 * (k + 1)],
                                 QT_sb[1][:, qs], start=False, stop=True)
            nc.scalar.activation(e_sb[k][:, q0:q0 + width], slot[:, 0:width],
                                 EXP, scale=1.0 / 16.0,
                                 accum_out=sparts[:, sidx:sidx + 1])

        def emit_row_scale(k, sparts):
            ssum = wpool.tile([128, 1], F32, name="ssum", tag="ssum")
            nc.vector.reduce_sum(ssum, sparts, axis=AX)
            rs = wpool.tile([128, 1], F32, name="rs", tag="rs")
            nc.vector.reciprocal(rs, ssum)
            nc.vector.tensor_scalar_mul(Vb_sb[k], Vb_sb[k], rs)

        # ============ phase 1a: scores k-tiles 0..7, big exp slots ==========
        with tc.tile_pool(name="psa", bufs=2, space="PSUM") as psa:
            for k in range(NK1):
                sparts = wpool.tile([128, 2], F32, name="sparts", tag="sparts")
                for half in range(2):
                    s = psa.tile([128, 2048], F32, name="psat", tag="psat")
                    emit_scores_slot(s, k, 2048 * half, 2048, sparts, half)
                emit_row_scale(k, sparts)

        # ====== phase 1b: scores k-tiles 8..15 + AV over keys 0..7 ==========
        with tc.tile_pool(name="psav", bufs=4, space="PSUM") as psav:

            def emit_av(pool, j, krange, accumulate_part):
                pa = pool.tile([128, D], F32, name="psavt", tag="psavt")
                for n, k in enumerate(krange):
                    nc.tensor.matmul(pa, e_sb[k][:, 128 * j:128 * (j + 1)],
                                     Vb_sb[k], start=(n == 0),
                                     stop=(n == len(krange) - 1))
                if not accumulate_part:
                    nc.vector.tensor_copy(part_sb[j], pa)
                else:
                    ot = wpool.tile([128, D], F32, name="osb", tag="osb")
                    nc.vector.tensor_tensor(ot, pa, part_sb[j], op=ADD)
                    nc.sync.dma_start(out[128 * j:128 * (j + 1), :], ot)

            with tc.tile_pool(name="psb", bufs=2, space="PSUM") as psb:
                for k in range(NK1, NKT):
                    sparts = wpool.tile([128, 4], F32, name="sparts",
                                        tag="sparts")
                    for quarter in range(4):
                        s = psb.tile([128, 1024], F32, name="psbt", tag="psbt")
                        emit_scores_slot(s, k, 1024 * quarter, 1024, sparts,
                                         quarter)
                    emit_row_scale(k, sparts)
                    for j in range(4 * (k - NK1), 4 * (k - NK1) + 4):
                        emit_av(psav, j, range(NK1), accumulate_part=False)

            # ========= phase 2: AV over keys 8..15 + partial add ===========
            with tc.tile_pool(name="psav2", bufs=4, space="PSUM") as psav2:
                for j in range(NQT):
                    pool = psav if j % 2 == 0 else psav2
                    emit_av(pool, j, range(NK1, NKT), accumulate_part=True)


def build():
    nc = bacc.Bacc("TRN2", target_bir_lowering=False, debug=False)
    xT = nc.dram_tensor("xT", [D, S], BF, kind="ExternalInput").ap()
    wpT = nc.dram_tensor("wpT", [D, 3 * D], BF, kind="ExternalInput").ap()
    bpack = nc.dram_tensor("bpack", [D, 2], F32, kind="ExternalInput").ap()
    bvr = nc.dram_tensor("bvr", [1, D], BF, kind="ExternalInput").ap()
    out = nc.dram_tensor("out", [S, D], F32, kind="ExternalOutput").ap()

    with tile.TileContext(nc) as tc:
        _emit(tc, xT, wpT, bpack, bvr, out)
    nc.compile()
    return nc


_NC = None


def _get_nc():
    global _NC
    if _NC is None:
        _NC = build()
    return _NC


def make_in_maps(x, Wq, bq, Wk, bk, Wv, bv):
    wpT = np.ascontiguousarray(
        np.concatenate([Wq.T, Wk.T, Wv.T], axis=1)).astype(BF16)
    bpack = np.ascontiguousarray(
        np.stack([np.asarray(bq, np.float32), np.asarray(bk, np.float32)],
                 axis=1))
    bvr = np.asarray(bv).reshape(1, D).astype(BF16)
    in_maps = []
    for core in range(NCORES):
        b, h = divmod(core, 2)
        xTb = np.asarray(x[b]).T.astype(BF16)
        if h:  # rotate so this core's keys are always columns 0:KH
            xTb = np.concatenate([xTb[:, KH:], xTb[:, :KH]], axis=1)
        in_maps.append({
            "xT": np.ascontiguousarray(xTb),
            "wpT": wpT, "bpack": bpack, "bvr": bvr,
        })
    return in_maps


def run(x, Wq, bq, Wk, bk, Wv, bv, trace=False):
    """Run on the 8 cores; returns (full_output, BassKernelResults)."""
    nc = _get_nc()
    in_maps = make_in_maps(x, Wq, bq, Wk, bk, Wv, bv)
    res = run_bass_kernel_spmd(nc, in_maps, core_ids=list(range(NCORES)),
                               trace=trace)
    parts = []
    for core in range(NCORES):
        p = res.results[core]["out"]
        if core % 2:  # undo the query rotation
            p = np.concatenate([p[KH:], p[:KH]], axis=0)
        parts.append(p)
    full = np.stack([parts[2 * b] + parts[2 * b + 1] for b in range(B)], axis=0)
    return full.astype(np.float32), res


def kernel(x, Wq, bq, Wk, bk, Wv, bv):
    full, _ = run(x, Wq, bq, Wk, bk, Wv, bv, trace=False)
    return full


# revision 8
# speedup vs baseline: 1.3361x; 1.0315x over previous
"""Trainium2 Bass kernel for an attention block with softmax over the QUERY axis.

Reference computation (per batch b):
    Q = x_b @ Wq^T + bq ; K = x_b @ Wk^T + bk ; V = x_b @ Wv^T + bv
    S = Q @ K^T / sqrt(256)
    attn = softmax(S, axis over queries)      # couples rows, not columns
    out_b = attn @ V

Sharding over 8 NeuronCores: core m handles batch b = m // 2 and the
key/value half h = m % 2.  The host passes x_b^T with the query axis
rotated so that the core's 2048 keys are always columns 0:2048; each core
holds the full query range for its batch, so the softmax over queries is
fully local.  Each core produces a partial output (sum over its 2048
keys); the host rotates back and adds the two partials per batch.  No
collectives are required.

On-core dataflow (all matmul operands bf16, fp32 accumulation):
    xT  (256d, 4096s)   host-transposed input, d on partitions
    QT  (256e, 4096q) = Wq @ x^T        KT (256e, 2048k) = Wk @ x_h^T
    V   (2048k, 256e) = x_h @ Wv^T      (natural layout, k on partitions)
    ST  (k, q) tiles  = KT^T slices @ QT (scores transposed, PSUM)
    e   = exp(ST/16)  on ACT, row sums via the activation accumulator
                      (no max-subtraction: |S/16| < ~2.2 for these inputs)
    V'  = V / s_k     per-partition scale (in place, DVE)
    out = e^T @ V'    k-split in half: the first half's matmuls are
                      interleaved under the ACT-bound scores phase.
"""

import numpy as np
import ml_dtypes

import concourse.bass as bass
import concourse.tile as tile
from concourse import bacc, mybir
from concourse.bass_utils import run_bass_kernel_spmd

BF16 = ml_dtypes.bfloat16
F32 = mybir.dt.float32
BF = mybir.dt.bfloat16

B, S, D = 4, 4096, 256
NCORES = 8
KH = S // 2          # 2048 keys per core
NKT = KH // 128      # 16 key tiles
NQT = S // 128       # 32 query tiles
NK1 = NKT // 2       # key tiles in the interleaved AV half

EXP = mybir.ActivationFunctionType.Exp
IDENT = mybir.ActivationFunctionType.Identity
AX = mybir.AxisListType.X
ADD = mybir.AluOpType.add


def _emit(tc, xT, wpT, bpack, bvr, out):
    nc = tc.nc

    with tc.tile_pool(name="const", bufs=1) as cpool, \
         tc.tile_pool(name="big", bufs=1) as bpool, \
         tc.tile_pool(name="work", bufs=4) as wpool:

        # ---- input loads: packed, few DMAs, earliest-needed first ----
        w_sb = [cpool.tile([128, 3 * D], BF, name=f"wsb{i}", tag=f"wsb{i}")
                for i in range(2)]
        bqk_sb = [cpool.tile([128, 2], F32, name=f"bqk{i}", tag=f"bqk{i}")
                  for i in range(2)]
        bv_sb = cpool.tile([1, D], BF, name="bv", tag="bv")
        ones = cpool.tile([1, 128], BF, name="ones", tag="ones")
        nc.vector.memset(ones, 1.0)

        # xT as one (128, 2*4096) tile: [:, 0:S] = d-rows 0:128, [:, S:2S] =
        # d-rows 128:256.  Loaded in three column-chunks so the projections
        # can start before the full tensor arrives.
        xT_sb = cpool.tile([128, 2 * S], BF, name="xTsb", tag="xTsb")
        xt_out = xT_sb.rearrange("p (t s) -> p t s", t=2)
        xt_in = xT.rearrange("(t p) s -> p t s", p=128)

        nc.sync.dma_start(bv_sb, bvr)
        nc.sync.dma_start(xt_out[:, :, 0:1024], xt_in[:, :, 0:1024])
        for i in range(2):
            nc.sync.dma_start(w_sb[i], wpT[128 * i:128 * (i + 1), :])
            nc.sync.dma_start(bqk_sb[i], bpack[128 * i:128 * (i + 1), :])
        nc.sync.dma_start(xt_out[:, :, 1024:KH], xt_in[:, :, 1024:KH])
        nc.sync.dma_start(xt_out[:, :, KH:S], xt_in[:, :, KH:S])
        xTs = [xT_sb[:, 0:S], xT_sb[:, S:2 * S]]

        def wq(i):
            return w_sb[i][:, 0:D]

        def wk(i):
            return w_sb[i][:, D:2 * D]

        def wv(i):
            return w_sb[i][:, 2 * D:3 * D]

        # ---- persistent tiles ----
        QT_sb = [bpool.tile([128, S], BF, name=f"QT{i}", tag=f"QT{i}")
                 for i in range(2)]
        KT_sb = [bpool.tile([128, KH], BF, name=f"KT{i}", tag=f"KT{i}")
                 for i in range(2)]
        Vb_sb = [bpool.tile([128, D], BF, name=f"Vb{k}", tag=f"Vb{k}")
                 for k in range(NKT)]
        e_sb = [bpool.tile([128, S], BF, name=f"e{k}", tag=f"e{k}")
                for k in range(NKT)]
        part_sb = [bpool.tile([128, D], BF, name=f"pt{j}", tag=f"pt{j}")
                   for j in range(NQT)]
        bvb_sb = cpool.tile([128, D], BF, name="bvb", tag="bvb")

        # ================= phase 0: projections (small-slot ring) ==========
        with tc.tile_pool(name="ps0", bufs=8, space="PSUM") as ps0:
            def slot0():
                return ps0.tile([128, 512], F32, name="ps0t", tag="ps0t")

            pt = slot0()
            nc.tensor.matmul(pt[:, 0:D], ones, bv_sb, start=True, stop=True)
            nc.vector.tensor_copy(bvb_sb, pt[:, 0:D])

            def emit_ktqt_group(dst, wsel, bcol, qb, i):
                # two d-tile matmuls per 512-column group; bias added on the
                # copyback (DVE for e-tile 0, ACT for e-tile 1)
                pt = slot0()
                cs = slice(512 * qb, 512 * (qb + 1))
                nc.tensor.matmul(pt, wsel(0)[:, 128 * i:128 * (i + 1)],
                                 xTs[0][:, cs], start=True, stop=False)
                nc.tensor.matmul(pt, wsel(1)[:, 128 * i:128 * (i + 1)],
                                 xTs[1][:, cs], start=False, stop=True)
                bias = bqk_sb[i][:, bcol:bcol + 1]
                if i == 0:
                    nc.vector.tensor_scalar_add(dst[i][:, cs], pt, bias)
                else:
                    nc.scalar.activation(dst[i][:, cs], pt, IDENT, bias=bias)

            def emit_v_group(k):
                # V natural layout; bv added via the broadcast tile on DVE
                pt = slot0()
                po = pt[:, 0:D]
                ks = slice(128 * k, 128 * (k + 1))
                nc.tensor.matmul(po, xTs[0][:, ks], wv(0),
                                 start=True, stop=False)
                nc.tensor.matmul(po, xTs[1][:, ks], wv(1),
                                 start=False, stop=True)
                nc.vector.tensor_tensor(Vb_sb[k], po, bvb_sb, op=ADD)

            # ordered by when their xT columns arrive: KT/V on the key half
            # first, QT (which also needs the query half) last
            for qb in range(2):
                for i in range(2):
                    emit_ktqt_group(KT_sb, wk, 1, qb, i)
            for k in range(8):
                emit_v_group(k)
            for qb in range(2, 4):
                for i in range(2):
                    emit_ktqt_group(KT_sb, wk, 1, qb, i)
            for k in range(8, NKT):
                emit_v_group(k)
            for qb in range(8):
                for i in range(2):
                    emit_ktqt_group(QT_sb, wq, 0, qb, i)

        # ---- shared helpers for the scores phases ----
        def emit_scores_slot(slot, k, q0, width, sparts, sidx):
            for g in range(width // 512):
                sub = slot[:, 512 * g:512 * (g + 1)]
                qs = slice(q0 + 512 * g, q0 + 512 * (g + 1))
                nc.tensor.matmul(sub, KT_sb[0][:, 128 * k:128 * (k + 1)],
                                 QT_sb[0][:, qs], start=True, stop=False)
                nc.tensor.matmul(sub, KT_sb[1][:, 128 * k:128 * (k + 1)],
                                 QT_sb[1][:, qs], start=False, stop=True)
            nc.scalar.activation(e_sb[k][:, q0:q0 + width], slot[:, 0:width],
                                 EXP, scale=1.0 / 16.0,
                                 accum_out=sparts[:, sidx:sidx + 1])

        def emit_row_scale(k, sparts):
            ssum = wpool.tile([128, 1], F32, name="ssum", tag="ssum")
            nc.vector.reduce_sum(ssum, sparts, axis=AX)
            rs = wpool.tile([128, 1], F32, name="rs", tag="rs")
            nc.vector.reciprocal(rs, ssum)
            nc.vector.tensor_scalar_mul(Vb_sb[k], Vb_sb[k], rs)

        # ============ phase 1a: scores k-tiles 0..7, big exp slots ==========
        with tc.tile_pool(name="psa", bufs=2, space="PSUM") as psa:
            for k in range(NK1):
                sparts = wpool.tile([128, 2], F32, name="sparts", tag="sparts")
                for half in range(2):
                    s = psa.tile([128, 2048], F32, name="psat", tag="psat")
                    emit_scores_slot(s, k, 2048 * half, 2048, sparts, half)
                emit_row_scale(k, sparts)

        # ====== phase 1b: scores k-tiles 8..15 + AV over keys 0..7 ==========
        with tc.tile_pool(name="psav", bufs=4, space="PSUM") as psav:

            def emit_av(pool, j, krange, accumulate_part):
                pa = pool.tile([128, D], F32, name="psavt", tag="psavt")
                for n, k in enumerate(krange):
                    nc.tensor.matmul(pa, e_sb[k][:, 128 * j:128 * (j + 1)],
                                     Vb_sb[k], start=(n == 0),
                                     stop=(n == len(krange) - 1))
                if not accumulate_part:
                    nc.vector.tensor_copy(part_sb[j], pa)
                else:
                    ot = wpool.tile([128, D], F32, name="osb", tag="osb")
                    nc.vector.tensor_tensor(ot, pa, part_sb[j], op=ADD)
                    nc.sync.dma_start(out[128 * j:128 * (j + 1), :], ot)

            with tc.tile_pool(name="psb", bufs=2, space="PSUM") as psb:
                for k in range(NK1, NKT):
                    sparts = wpool.tile([128, 4], F32, name="sparts",
                                        tag="sparts")
                    for quarter in range(4):
                        s = psb.tile([128, 1024], F32, name="psbt", tag="psbt")
                        emit_scores_slot(s, k, 1024 * quarter, 1024, sparts,
                                         quarter)
                    emit_row_scale(k, sparts)
                    for j in range(4 * (k - NK1), 4 * (k - NK1) + 4):
                        emit_av(psav, j, range(NK1), accumulate_part=False)

            # ========= phase 2: AV over keys 8..15 + partial add ===========
            with tc.tile_pool(name="psav2", bufs=4, space="PSUM") as psav2:
                for j in range(NQT):
                    pool = psav if j % 2 == 0 else psav2
                    emit_av(pool, j, range(NK1, NKT), accumulate_part=True)


def build():
    nc = bacc.Bacc("TRN2", target_bir_lowering=False, debug=False)
    xT = nc.dram_tensor("xT", [D, S], BF, kind="ExternalInput").ap()
    wpT = nc.dram_tensor("wpT", [D, 3 * D], BF, kind="ExternalInput").ap()
    bpack = nc.dram_tensor("bpack", [D, 2], F32, kind="ExternalInput").ap()
    bvr = nc.dram_tensor("bvr", [1, D], BF, kind="ExternalInput").ap()
    out = nc.dram_tensor("out", [S, D], F32, kind="ExternalOutput").ap()

    with tile.TileContext(nc) as tc:
        _emit(tc, xT, wpT, bpack, bvr, out)
    nc.compile()
    return nc


_NC = None


def _get_nc():
    global _NC
    if _NC is None:
        _NC = build()
    return _NC


def make_in_maps(x, Wq, bq, Wk, bk, Wv, bv):
    wpT = np.ascontiguousarray(
        np.concatenate([Wq.T, Wk.T, Wv.T], axis=1)).astype(BF16)
    bpack = np.ascontiguousarray(
        np.stack([np.asarray(bq, np.float32), np.asarray(bk, np.float32)],
                 axis=1))
    bvr = np.asarray(bv).reshape(1, D).astype(BF16)
    in_maps = []
    for core in range(NCORES):
        b, h = divmod(core, 2)
        xTb = np.asarray(x[b]).T.astype(BF16)
        if h:  # rotate so this core's keys are always columns 0:KH
            xTb = np.concatenate([xTb[:, KH:], xTb[:, :KH]], axis=1)
        in_maps.append({
            "xT": np.ascontiguousarray(xTb),
            "wpT": wpT, "bpack": bpack, "bvr": bvr,
        })
    return in_maps


def run(x, Wq, bq, Wk, bk, Wv, bv, trace=False):
    """Run on the 8 cores; returns (full_output, BassKernelResults)."""
    nc = _get_nc()
    in_maps = make_in_maps(x, Wq, bq, Wk, bk, Wv, bv)
    res = run_bass_kernel_spmd(nc, in_maps, core_ids=list(range(NCORES)),
                               trace=trace)
    parts = []
    for core in range(NCORES):
        p = res.results[core]["out"]
        if core % 2:  # undo the query rotation
            p = np.concatenate([p[KH:], p[:KH]], axis=0)
        parts.append(p)
    full = np.stack([parts[2 * b] + parts[2 * b + 1] for b in range(B)], axis=0)
    return full.astype(np.float32), res


def kernel(x, Wq, bq, Wk, bk, Wv, bv):
    full, _ = run(x, Wq, bq, Wk, bk, Wv, bv, trace=False)
    return full
